# revision 21
# baseline (speedup 1.0000x reference)
"""Trainium2 Bass kernel for nn_LocalContextProcessor (local linear attention).

Computation (per 256-token window, fully independent):
    qkv = x @ W_qkv ; q,k,v split ; per head: q,k <- elu(.)+1
    ctx = k^T @ v ; attn = (q @ ctx) ; out = attn @ W_out + b_out

Sharding: data-parallel over the 64 windows (4 batch x 16 windows);
each of the 8 cores processes 8 consecutive windows (2048 tokens).
Weights are replicated to every core.

Per-core dataflow (all matmuls contract over the partition dim):
    x natural (n,d) --PE transpose--> x_T (d,n)
    q_T (j,n)  = [W_qkv chunk as lhsT]  @ x_T      (float32r, N=256)
    k,v (n,j)  = [x_T chunk as lhsT]    @ W_qkv    (float32r, N=512)
    elu+1 on q_T and k:  exp(min(x,0)) + relu(x)
    ctx (d,e)  = [k head as lhsT] @ v head          (accumulate 2 n-chunks)
    attnT (e,n)= [ctx as lhsT]    @ q_T head        (float32r, N=256)
    out (n,c)  = [attnT chunk as lhsT] @ W_out + b  (float32r, N=512)
"""

import numpy as np

P = 128
WS = 256          # window size
NW = 8            # windows per core
TOK = WS * NW     # 2048 tokens per core
D = 1024
J3 = 3 * D        # qkv width
H = 8
DH = 128
NCORES = 8

_CACHE = {}


def _build_nc(finalize=True, reps=1):
    import concourse.bass as bass
    import concourse.tile as tile
    from concourse import bacc, mybir
    from concourse.masks import make_identity
    from contextlib import ExitStack

    f32 = mybir.dt.float32
    f32r = mybir.dt.float32r
    AF = mybir.ActivationFunctionType

    nc = bacc.Bacc()
    x_d = nc.declare_dram_parameter("x", [TOK, D], f32, isOutput=False)
    wq_d = nc.declare_dram_parameter("w_qkv", [D, J3], f32r, isOutput=False)
    wo_d = nc.declare_dram_parameter("w_out", [D, D], f32r, isOutput=False)
    b_d = nc.declare_dram_parameter("b_out", [D], f32, isOutput=False)
    out_d = nc.declare_dram_parameter("out", [TOK, D], f32, isOutput=True)

    with ExitStack() as ctx:
        tc = ctx.enter_context(tile.TileContext(nc))
        consts = ctx.enter_context(tc.tile_pool(name="consts", bufs=1))
        io2 = ctx.enter_context(tc.tile_pool(name="io2", bufs=2))
        work = ctx.enter_context(tc.tile_pool(name="work", bufs=1))
        tmps = ctx.enter_context(tc.tile_pool(name="tmps", bufs=2))
        obp = ctx.enter_context(tc.tile_pool(name="obp", bufs=2))
        ps_mm = ctx.enter_context(tc.tile_pool(name="ps_mm", bufs=5, space="PSUM"))
        ps_tr = ctx.enter_context(tc.tile_pool(name="ps_tr", bufs=1, space="PSUM"))
        ps_ctx = ctx.enter_context(tc.tile_pool(name="ps_ctx", bufs=2, space="PSUM"))

        # ---- window-0 x load + identity first, so PE transposes can start
        # while the (much larger) weight DMAs stream in ----
        xn0_box = [None]
        xn0_box[0] = io2.tile([P, 2, D], f32, tag="xn", bufs=1, name="xn0")
        nc.sync.dma_start(
            out=xn0_box[0][:],
            in_=x_d[0:WS, :].rearrange("(i p) d -> p i d", p=P))
        ident = consts.tile([P, P], f32)
        make_identity(nc, ident[:])

        # ---- resident constants; W_qkv chunked by column so the first
        # q/k/v matmuls only gate on their slice, not the full 12MB ----
        w_sb = consts.tile([P, 8, J3], f32r)        # W_qkv: part=d%128, mid=d//128
        wq_r = wq_d[:, :].rearrange("(c p) j -> p c j", p=P)
        for s in range(6):
            nc.sync.dma_start(out=w_sb[:, :, s * 512:(s + 1) * 512],
                              in_=wq_r[:, :, s * 512:(s + 1) * 512])
        wo_sb = consts.tile([P, 8, D], f32r)        # W_out: part=i%128, mid=i//128
        wo_r = wo_d[:, :].rearrange("(c p) j -> p c j", p=P)
        for s in range(2):
            nc.sync.dma_start(out=wo_sb[:, :, s * 512:(s + 1) * 512],
                              in_=wo_r[:, :, s * 512:(s + 1) * 512])
        bias_sb = consts.tile([P, D], f32)          # b_out replicated on partitions
        b_ap = b_d[:]
        bias_bcast = bass.AP(tensor=b_ap.tensor, offset=b_ap.offset,
                             ap=[[0, P]] + list(b_ap.ap))
        nc.sync.dma_start(out=bias_sb[:], in_=bias_bcast)

        # Software-pipelined emission: stage_a(w+1) (transpose + q matmuls) is
        # emitted between stage_m(w) (k/v matmuls) and stage_b(w) (attention +
        # out-proj) so the PE has dense work while window w's k-elu drains on
        # DVE/ACT.  qt/xt are double-buffered to allow the overlap.
        state = {}

        def stage_tr(w):
            if w == 0 and xn0_box[0] is not None:
                xn = xn0_box[0]
                xn0_box[0] = None
            else:
                xn = io2.tile([P, 2, D], f32, tag="xn", bufs=1)
                nc.sync.dma_start(
                    out=xn[:],
                    in_=x_d[w * WS:(w + 1) * WS, :].rearrange("(i p) d -> p i d", p=P))
            xt = io2.tile([P, 8, WS], f32r, tag="xt")   # part=d%128, mid=d//128
            for dc in range(8):
                for i in range(2):
                    trp = ps_tr.tile([P, P], f32, tag="tr")
                    nc.tensor.transpose(trp[:], xn[:, i, dc * P:(dc + 1) * P], ident[:])
                    nc.scalar.copy(out=xt[:, dc, i * P:(i + 1) * P], in_=trp[:])
            state[w] = [xt]

        def stage_q(w):
            xt = state[w][0]
            # q_T = elu(W_q^T x^T)+1 : (j,n)
            qt = io2.tile([P, 8, WS], f32r, tag="qt", bufs=1)
            for jc in range(8):
                qp = ps_mm.tile([P, WS], f32, tag="mm")
                for dc in range(8):
                    nc.tensor.matmul(qp[:], lhsT=w_sb[:, dc, jc * P:(jc + 1) * P],
                                     rhs=xt[:, dc, :],
                                     start=(dc == 0), stop=(dc == 7))
                t1 = tmps.tile([P, 512], f32, tag="t1", bufs=1)
                t2 = tmps.tile([P, 512], f32, tag="t2")
                nc.vector.tensor_scalar_min(t1[:, :WS], qp[:], 0.0)
                nc.scalar.activation(t2[:, :WS], t1[:, :WS], AF.Exp)
                nc.scalar.activation(qt[:, jc, :], qp[:], AF.Relu)
                nc.vector.tensor_add(qt[:, jc, :], qt[:, jc, :], t2[:, :WS])
            state[w].append(qt)

        def stage_m(w):
            xt = state[w][0]
            kn = work.tile([P, 2, D], f32r, tag="kn")
            vn = work.tile([P, 2, D], f32r, tag="vn")
            for i in range(2):
                for jc in range(4):   # 4 x 512 across [k | v]
                    kvp = ps_mm.tile([P, 512], f32, tag="mm")
                    for dc in range(8):
                        nc.tensor.matmul(
                            kvp[:], lhsT=xt[:, dc, i * P:(i + 1) * P],
                            rhs=w_sb[:, dc, D + jc * 512:D + (jc + 1) * 512],
                            start=(dc == 0), stop=(dc == 7))
                    if jc < 2:  # k columns: elu+1
                        dst = kn[:, i, jc * 512:(jc + 1) * 512]
                        t1 = tmps.tile([P, 512], f32, tag="t1", bufs=1)
                        t2 = tmps.tile([P, 512], f32, tag="t2")
                        nc.vector.tensor_scalar_min(t1[:], kvp[:], 0.0)
                        nc.scalar.activation(t2[:], t1[:], AF.Exp)
                        nc.scalar.activation(dst, kvp[:], AF.Relu)
                        nc.vector.tensor_add(dst, dst, t2[:])
                    else:       # v columns: plain copy
                        nc.scalar.copy(out=vn[:, i, (jc - 2) * 512:(jc - 1) * 512],
                                       in_=kvp[:])
            state[w] += [kn, vn]

        def stage_b(w):
            _, qt, kn, vn = state.pop(w)
            # ctx = k_h^T @ v: pair heads so the moving operand is 256 wide
            # (f32r runs 1 cyc/row at N>=256 vs 4 at N=128); half of each
            # product is discarded but net PE time is still 2x lower.
            ctxs = work.tile([P, H, DH], f32r, tag="ctxs")   # (d, h, e)
            at = work.tile([P, H, WS], f32r, tag="at")       # (e, h, n)
            for h in range(H):
                vlo = (h // 2) * 2 * DH   # start of this head-pair's v columns
                cp = ps_ctx.tile([P, 2 * DH], f32, tag="cps")
                for i in range(2):
                    nc.tensor.matmul(cp[:], lhsT=kn[:, i, h * DH:(h + 1) * DH],
                                     rhs=vn[:, i, vlo:vlo + 2 * DH],
                                     start=(i == 0), stop=(i == 1))
                off = (h % 2) * DH        # which half of cp is this head's ctx
                nc.vector.tensor_copy(ctxs[:, h, :], cp[:, off:off + DH])
                ap_ = ps_mm.tile([P, WS], f32, tag="mm")
                nc.tensor.matmul(ap_[:], lhsT=ctxs[:, h, :], rhs=qt[:, h, :],
                                 start=True, stop=True)
                nc.scalar.copy(out=at[:, h, :], in_=ap_[:])

            # output projection; bias added in PSUM, DMA straight from PSUM
            for i in range(2):
                for cc in range(2):
                    op = ps_mm.tile([P, 512], f32, tag="mm")
                    for hc in range(8):
                        nc.tensor.matmul(op[:], lhsT=at[:, hc, i * P:(i + 1) * P],
                                         rhs=wo_sb[:, hc, cc * 512:(cc + 1) * 512],
                                         start=(hc == 0), stop=(hc == 7))
                    ob = obp.tile([P, 512], f32, tag="ob")
                    nc.vector.tensor_add(ob[:], op[:],
                                         bias_sb[:, cc * 512:(cc + 1) * 512])
                    nc.sync.dma_start(
                        out=out_d[w * WS + i * P: w * WS + (i + 1) * P,
                                  cc * 512:(cc + 1) * 512],
                        in_=ob[:])

        for _rep in range(reps):
            stage_tr(0)
            stage_q(0)
            stage_m(0)
            for w in range(1, NW):
                stage_tr(w)
                stage_b(w - 1)
                stage_q(w)
                stage_m(w)
            stage_b(NW - 1)
    if finalize:
        nc.finalize()
    return nc


def _get_nc():
    if "nc" not in _CACHE:
        _CACHE["nc"] = _build_nc()
    return _CACHE["nc"]


def kernel(x, W_qkv, W_out, b_out):
    from concourse.bass_utils import run_bass_kernel_spmd

    nc = _get_nc()
    x = np.ascontiguousarray(np.asarray(x, dtype=np.float32))
    W_qkv = np.ascontiguousarray(np.asarray(W_qkv, dtype=np.float32))
    W_out = np.ascontiguousarray(np.asarray(W_out, dtype=np.float32))
    b_out = np.ascontiguousarray(np.asarray(b_out, dtype=np.float32))

    b, n, d = x.shape
    xf = x.reshape(b * n, d)
    in_maps = [
        {"x": xf[c * TOK:(c + 1) * TOK], "w_qkv": W_qkv, "w_out": W_out,
         "b_out": b_out}
        for c in range(NCORES)
    ]
    res = run_bass_kernel_spmd(nc, in_maps, list(range(NCORES)))
    out = np.concatenate([res.results[c]["out"] for c in range(NCORES)], axis=0)
    return out.reshape(b, n, d)
